# revision 19
# baseline (speedup 1.0000x reference)
"""Bayesian LSTM (variational dropout) forward on 8 Trainium2 NeuronCores.

Strategy: tensor-parallel over the hidden/gate dimension (8 x 128 hidden
units per core), full batch on every core.  The serial T=512 recurrence
runs fully unrolled in raw Bass with a per-step all-gather of the masked
hidden state done via cross-core SBUF->SBUF remote DMA (XOR-slot scheme,
single-dest remote_dma_broadcast per peer).  The input projection
x_t @ W_ih^T (+bias) is fused into each step's PSUM accumulation as extra
matmuls whose stationary operand is the pre-transposed x tile; they are
emitted ahead of the arrival wait so they fill TensorE during the previous
step's activation chain.  Gate columns are ordered [i | f | o | g] and the
matmuls are split into two 256-column sections in separate PSUM banks so
sigmoid(i,f) runs while the [o,g] section is still accumulating.  All
matmul operands are float32r (full-rate PE, fp32 bytes).  ScalarE applies
sigmoid/tanh straight from PSUM, VectorE forms c_t and the masked
transposed h_t (the dropout masks are folded into transposed-sigma(o)
pre-products), TensorE transposes c_t and sigma(o) via identity matmuls,
and GpSimd fires the 7 peer broadcasts.

Slot d of the h^T gather buffer holds the chunk of core (me XOR d);
weights are permuted host-side to match, which keeps every access pattern
compile-time static under SPMD.  Cross-die XOR offsets (>=4) use wire
Delta^2 to compensate the logical->physical NC die-flip.
"""

import os
import sys

sys.path.insert(0, "/opt/trn_rl_repo")

import numpy as np

B, T, D, H = 64, 512, 512, 1024
NC = 8
HC = H // NC          # hidden units per core (128)
G = 4 * HC            # gate columns per core (512)
KD = D // 128         # x contraction chunks (4)
KH = H // 128         # h contraction chunks (8)

# logical XOR offset -> wire Delta-tpb (measured: cross-die hops land with
# an extra XOR 2 on this platform's logical->physical NC map)
WIRE = {d: (d if d < 4 else d ^ 2) for d in range(1, NC)}

_DT_STR = os.environ.get("LSTM_KERNEL_DT", "float32")


def _dt():
    import concourse.mybir as mybir

    return {"float32": mybir.dt.float32, "bfloat16": mybir.dt.bfloat16}[_DT_STR]


def _np_dt():
    return {"float32": np.float32, "bfloat16": None}[_DT_STR]


def _patch_libnrt():
    # The axon client has no neuron driver; these id maps are only used by
    # client-side plumbing (the NEFF itself uses relative routing).
    try:
        import concourse.libnrt as libnrt

        libnrt.get_trn2_nc_mapping()
    except Exception:
        import concourse.libnrt as libnrt

        libnrt.get_trn2_nc_mapping = lambda: {
            (d, i): i for d in range(16) for i in range(8)
        }
        libnrt.get_device_id_to_routing_id_mapping = lambda: {
            i: i for i in range(16)
        }


def build(wire=None, dt_str=None, n_steps=None):
    """Build the SPMD Bass program (identical on all 8 cores)."""
    global _DT_STR
    if dt_str is not None:
        _DT_STR = dt_str
    wire = wire or WIRE
    TS = n_steps or T

    import concourse.bacc as bacc
    import concourse.mybir as mybir

    F32 = mybir.dt.float32
    ACTF = mybir.ActivationFunctionType
    # float32r: full-rate PE mode, 4-byte fp32 operand bytes.  Matmul
    # operand tensors are declared float32r end-to-end so walrus's
    # "rounded to FP32r" producer check passes.
    DT = mybir.dt.float32r if _DT_STR == "float32" else _dt()
    bc = lambda ap: ap

    nc = bacc.Bacc("TRN2", target_bir_lowering=False)

    # ---- DRAM (per-core inputs, host-prepared) ----
    x_dram = nc.dram_tensor("x_steps", (TS, 128, KD * B), DT, kind="ExternalInput")
    whh_dram = nc.dram_tensor("whh", (KH, 128, G), DT, kind="ExternalInput")
    wih_dram = nc.dram_tensor("wih", (KD, 128, G), DT, kind="ExternalInput")
    bias_dram = nc.dram_tensor("bias", (1, G), DT, kind="ExternalInput")
    ones_dram = nc.dram_tensor("ones", (1, B), DT, kind="ExternalInput")
    ident_dram = nc.dram_tensor("ident", (B, B), F32, kind="ExternalInput")
    hmt_dram = nc.dram_tensor("hid_maskT", (HC, B), F32, kind="ExternalInput")
    omt_dram = nc.dram_tensor("out_maskT", (HC, B), F32, kind="ExternalInput")

    y_dram = nc.dram_tensor("y", (TS, HC, B), F32, kind="ExternalOutput")
    hf_dram = nc.dram_tensor("hf", (HC, B), F32, kind="ExternalOutput")
    cf_dram = nc.dram_tensor("cf", (B, HC), F32, kind="ExternalOutput")

    # ---- SBUF ----
    sb = nc.alloc_sbuf_tensor
    whh_sb = sb("whh_sb", (128, KH * G), DT)
    wih_sb = sb("wih_sb", (128, KD * G), DT)
    bias_sb = sb("bias_sb", (1, G), DT)
    ones_sb = sb("ones_sb", (1, B), DT)
    ident_sb = sb("ident_sb", (B, B), F32)
    hmt_sb = sb("hmt_sb", (HC, B), F32)
    omt_sb = sb("omt_sb", (HC, B), F32)
    x_sb = [sb(f"x_sb{p}", (128, KD * B), DT) for p in range(2)]
    buf = [sb(f"hbuf{p}", (128, NC * B), DT) for p in range(2)]
    sig_sb = [sb(f"sig{p}", (B, 3 * HC), F32) for p in range(2)]
    g_sb = [sb(f"g{p}", (B, HC), F32) for p in range(2)]
    c_sb = [sb(f"c{p}", (B, HC), F32) for p in range(2)]
    tmp1_sb = [sb(f"tmp1{p}", (B, HC), F32) for p in range(2)]
    tmp2_sb = [sb(f"tmp2{p}", (B, HC), F32) for p in range(2)]
    tanhT_sb = [sb(f"tanhT{p}", (HC, B), F32) for p in range(2)]
    om_sb = [sb(f"om{p}", (HC, B), F32) for p in range(2)]
    oym_sb = [sb(f"oym{p}", (HC, B), F32) for p in range(2)]
    yT_sb = [sb(f"yT{p}", (HC, B), F32) for p in range(2)]

    ps = [nc.alloc_psum_tensor(f"ps{p}", (B, 2 * HC), F32) for p in range(2)]
    ps2 = [nc.alloc_psum_tensor(f"ps2{p}", (B, 2 * HC), F32) for p in range(2)]
    cT_ps = nc.alloc_psum_tensor("cT_ps", (HC, B), F32)
    oT_ps = nc.alloc_psum_tensor("oT_ps", (HC, B), F32)

    # ---- semaphores ----
    sem = nc.alloc_semaphore
    ld_sem = sem("ld_sem")
    x_sems = [sem("x_sem0"), sem("x_sem1")]
    mm_sem = sem("mm_sem")
    m1_sem = sem("m1_sem")
    asem = sem("asem")
    a2_sem = sem("a2_sem")
    dsem = sem("dsem")
    tp_sem = sem("tp_sem")
    csem = sem("csem")
    prep_sem = sem("prep_sem")
    rsem = [sem("rsem0"), sem("rsem1")]
    lsem = [sem("lsem0"), sem("lsem1")]
    yv_sem = sem("yv_sem")
    yd_sems = [sem("yd_sem0"), sem("yd_sem1")]
    fin_sem = sem("fin_sem")
    pxr_sem = sem("pxr_sem")
    init_sem = sem("init_sem")

    N_LOADS = KH + KD + 5  # prologue dmas: whh, wih, bias, ones, ident, 2 masks

    with nc.Block() as block:

        @block.sync
        def _(sync):
            for k in range(KH):
                sync.dma_start(
                    whh_sb[:, k * G:(k + 1) * G], whh_dram[k, :, :]
                ).then_inc(ld_sem, 16)
            for k in range(KD):
                sync.dma_start(
                    wih_sb[:, k * G:(k + 1) * G], wih_dram[k, :, :]
                ).then_inc(ld_sem, 16)
            sync.dma_start(bias_sb[:, :], bias_dram[:, :]).then_inc(ld_sem, 16)
            sync.dma_start(ones_sb[:, :], ones_dram[:, :]).then_inc(ld_sem, 16)
            sync.dma_start(ident_sb[:, :], ident_dram[:, :]).then_inc(ld_sem, 16)
            sync.dma_start(hmt_sb[:, :], hmt_dram[:, :]).then_inc(ld_sem, 16)
            sync.dma_start(omt_sb[:, :], omt_dram[:, :]).then_inc(ld_sem, 16)
            sync.dma_start(x_sb[0][:, :], x_dram[0, :, :]).then_inc(x_sems[0], 16)
            for t in range(TS):
                if t + 1 < TS:
                    # prefetch x tile for step t+1 (x-MMs of t-1 released it)
                    if t == 1:
                        sync.wait_ge(mm_sem, 1)
                    elif t >= 2:
                        sync.wait_ge(pxr_sem, t - 1)
                    sync.dma_start(
                        x_sb[(t + 1) % 2][:, :], x_dram[t + 1, :, :]
                    ).then_inc(x_sems[(t + 1) % 2], 16)
                sync.wait_ge(yv_sem, t + 1)
                sync.dma_start(y_dram[t, :, :], yT_sb[t % 2][:, :]).then_inc(
                    yd_sems[t % 2], 16
                )
            sync.wait_ge(csem, TS)
            sync.dma_start(
                hf_dram[:, :], buf[(TS - 1) % 2][:, 0:B].bitcast(F32)
            ).then_inc(fin_sem, 16)
            sync.dma_start(cf_dram[:, :], c_sb[(TS - 1) % 2][:, :]).then_inc(
                fin_sem, 16
            )
            sync.wait_ge(fin_sem, 32)

        @block.tensor
        def _(tensor):
            tensor.wait_ge(ld_sem, 16 * N_LOADS)
            HG = 2 * HC  # half-section width (256)
            for t in range(TS):
                par = t % 2

                def xbias(lo, is_first, t=t, par=par):
                    # bias + x matmuls for gate columns [lo, lo+HG)
                    bank = ps[par] if lo == 0 else ps2[par]
                    tensor.matmul(
                        bank[:, :], bc(ones_sb[:, :]),
                        bc(bias_sb[:, lo:lo + HG]),
                        start=True, stop=False, skip_group_check=True,
                    )
                    for k in range(KD):
                        mm = tensor.matmul(
                            bank[:, :],
                            bc(x_sb[par][:, k * B:(k + 1) * B]),
                            bc(wih_sb[:, k * G + lo:k * G + lo + HG]),
                            start=False,
                            stop=(t == 0 and k == KD - 1),
                            skip_group_check=True,
                        )
                    if is_first and t >= 1:
                        mm.then_inc(pxr_sem, 1)
                    return mm


                def hmms(lo, t=t, par=par):
                    bank = ps[par] if lo == 0 else ps2[par]
                    for d in range(KH):
                        mm = tensor.matmul(
                            bank[:, :],
                            bc(buf[(t - 1) % 2][:, d * B:(d + 1) * B]),
                            bc(whh_sb[:, d * G + lo:d * G + lo + HG]),
                            start=False, stop=(d == KH - 1),
                            skip_group_check=True,
                        )
                    return mm

                if t == 0:
                    # step-0 gates have no h term; x/bias close the groups
                    tensor.wait_ge(x_sems[0], 16)
                    xbias(0, True, t=0, par=0).then_inc(m1_sem, 1)
                    xbias(HG, False, t=0, par=0).then_inc(mm_sem, 1)
                else:
                    # ---- h terms: need all 7 peer chunks + own slot0 ----
                    tensor.wait_ge(rsem[(t - 1) % 2], 14 * ((t - 1) // 2 + 1))
                    tensor.wait_ge(csem, t)
                    hmms(0).then_inc(m1_sem, 1)
                    hmms(HG).then_inc(mm_sem, 1)
                if t + 1 < TS:
                    # next step's x/bias terms fill the PE gap in this chain
                    tensor.wait_ge(x_sems[(t + 1) % 2], 16 * ((t + 1) // 2 + 1))
                    xbias(0, True, t=t + 1, par=(t + 1) % 2)
                    xbias(HG, False, t=t + 1, par=(t + 1) % 2)
                # ---- transposes of this step's chain values ----
                tensor.wait_ge(asem, 3 * t + 3)  # sigma(o) done
                tensor.transpose(
                    oT_ps[:, :], sig_sb[par][:, 2 * HC:3 * HC], ident_sb[:, :]
                ).then_inc(tp_sem, 1)
                tensor.wait_ge(dsem, t + 1)
                tensor.transpose(
                    cT_ps[:, :], c_sb[par][:, :], ident_sb[:, :]
                ).then_inc(tp_sem, 1)

        @block.scalar
        def _(scalar):
            for t in range(TS):
                par = t % 2
                scalar.wait_ge(m1_sem, t + 1)
                # sigmoid over [i | f], straight from PSUM (half 1)
                scalar.activation(
                    sig_sb[par][:, 0:2 * HC], ps[par][:, :], ACTF.Sigmoid
                ).then_inc(asem, 1)
                scalar.wait_ge(mm_sem, t + 1)
                scalar.activation(
                    g_sb[par][:, :], ps2[par][:, HC:2 * HC], ACTF.Tanh
                ).then_inc(asem, 1)
                scalar.activation(
                    sig_sb[par][:, 2 * HC:3 * HC], ps2[par][:, 0:HC],
                    ACTF.Sigmoid,
                ).then_inc(asem, 1)
                scalar.wait_ge(tp_sem, 2 * t + 2)
                scalar.activation(
                    tanhT_sb[par][:, :], cT_ps[:, :], ACTF.Tanh
                ).then_inc(a2_sem, 1)

        @block.vector
        def _(vector):
            vector.memset(c_sb[1][:, :], 0.0).then_inc(init_sem, 1)
            for t in range(TS):
                par = t % 2
                vector.wait_ge(asem, 3 * t + 1)
                if t == 0:
                    vector.wait_ge(init_sem, 1)
                # t2 = sigma(f) * c_{t-1}
                vector.tensor_mul(
                    tmp2_sb[par][:, :], sig_sb[par][:, HC:2 * HC],
                    c_sb[1 - par][:, :],
                )
                vector.wait_ge(asem, 3 * t + 2)
                # t1 = sigma(i) * tanh(g)
                vector.tensor_mul(
                    tmp1_sb[par][:, :], sig_sb[par][:, 0:HC], g_sb[par][:, :]
                )
                vector.drain()
                vector.tensor_add(
                    c_sb[par][:, :], tmp1_sb[par][:, :], tmp2_sb[par][:, :]
                ).then_inc(dsem, 1)
                # pre-masked sigma(o)^T products (oT ready well before tanh)
                vector.wait_ge(tp_sem, 2 * t + 1)
                vector.tensor_mul(om_sb[par][:, :], oT_ps[:, :], hmt_sb[:, :])
                vector.tensor_mul(oym_sb[par][:, :], oT_ps[:, :], omt_sb[:, :])
                vector.drain()
                # masked h^T -> own slot 0 of the gather buffer
                vector.wait_ge(a2_sem, t + 1)
                if t >= 2:
                    vector.wait_ge(lsem[par], 112 * (t // 2))
                vector.tensor_mul(
                    buf[par][:, 0:B], om_sb[par][:, :], tanhT_sb[par][:, :]
                ).then_inc(csem, 1)
                # y_t^T = h_t^T * out_mask^T
                if t >= 2:
                    vector.wait_ge(yd_sems[t % 2], 16 * (t // 2))
                vector.tensor_mul(
                    yT_sb[par][:, :], oym_sb[par][:, :], tanhT_sb[par][:, :]
                ).then_inc(yv_sem, 1)

        @block.gpsimd
        def _(gpsimd):
            for t in range(TS - 1):  # nobody needs h_{T-1}
                par = t % 2
                for d in range(1, NC):
                    rdests = [None] * NC
                    rdests[d] = (0, wire[d])
                    gpsimd.remote_dma_broadcast(
                        buf[par][:, d * B:(d + 1) * B],
                        buf[par][:, 0:B],
                        rsem[par],
                        lsem[par],
                        rdests=rdests,
                    ).then_inc(prep_sem, 1)
                gpsimd.wait_ge(prep_sem, 7 * (t + 1))
                gpsimd.wait_ge(csem, t + 1)
                gpsimd.trigger_dma(count=7)
            # drain my sends before program end
            gpsimd.wait_ge(lsem[0], 112 * ((TS - 1 + 1) // 2))
            gpsimd.wait_ge(lsem[1], 112 * ((TS - 1) // 2))

    nc.compile()
    return nc


def _prep_inputs(x, W_ih, W_hh, b_ih, b_hh, in_mask, hid_mask, out_mask,
                 n_steps=None):
    """Host-side sharding/layout prep -> per-core in_maps."""
    TS = n_steps or T
    np_dt = np.float32 if _DT_STR == "float32" else None
    try:
        import ml_dtypes

        bf16 = ml_dtypes.bfloat16
    except ImportError:
        bf16 = None
    cast = (lambda a: a.astype(np.float32)) if _DT_STR == "float32" else (
        lambda a: a.astype(bf16)
    )

    x = np.asarray(x, np.float32)
    W_ih = np.asarray(W_ih, np.float32)
    W_hh = np.asarray(W_hh, np.float32)
    b = np.asarray(b_ih, np.float32) + np.asarray(b_hh, np.float32)
    in_mask = np.asarray(in_mask, np.float32)
    hid_mask = np.asarray(hid_mask, np.float32)
    out_mask = np.asarray(out_mask, np.float32)

    x0 = x.copy()
    x0[:, 0, :] *= in_mask  # reference quirk: input mask on t=0 only

    # x_steps[t, p, k*B + b] = x0[b, t, 128k + p]
    x0 = x0[:, :TS, :]
    xs = x0.transpose(1, 2, 0).reshape(TS, KD, 128, B)
    x_steps = np.ascontiguousarray(xs.transpose(0, 2, 1, 3)).reshape(TS, 128, KD * B)
    x_steps = cast(x_steps)

    ident = np.eye(B, dtype=np.float32)
    ones = cast(np.ones((1, B), np.float32))

    in_maps = []
    for r in range(NC):
        hr = slice(r * HC, (r + 1) * HC)
        idx = np.concatenate([
            np.arange(0 * H + r * HC, 0 * H + (r + 1) * HC),   # i
            np.arange(1 * H + r * HC, 1 * H + (r + 1) * HC),   # f
            np.arange(3 * H + r * HC, 3 * H + (r + 1) * HC),   # o
            np.arange(2 * H + r * HC, 2 * H + (r + 1) * HC),   # g
        ])
        wihT = np.ascontiguousarray(W_ih[idx, :].T)           # [D, G]
        whhT = np.ascontiguousarray(W_hh[idx, :].T)           # [H, G]
        whh_slots = np.stack(
            [whhT[(r ^ d) * 128:((r ^ d) + 1) * 128, :] for d in range(KH)]
        )                                                      # [8, 128, G]
        in_maps.append({
            "x_steps": x_steps,
            "whh": cast(whh_slots),
            "wih": cast(wihT.reshape(KD, 128, G)),
            "bias": cast(b[idx].reshape(1, G)),
            "ones": ones,
            "ident": ident,
            "hid_maskT": np.ascontiguousarray(hid_mask[:, hr].T),
            "out_maskT": np.ascontiguousarray(out_mask[:, hr].T),
        })
    return in_maps


def _assemble(results, n_steps=None):
    TS = n_steps or T
    y = np.empty((B, TS, H), np.float32)
    h_f = np.empty((B, H), np.float32)
    c_f = np.empty((B, H), np.float32)
    for r, res in enumerate(results):
        hr = slice(r * HC, (r + 1) * HC)
        y[:, :, hr] = res["y"].transpose(2, 0, 1)      # [T,HC,B] -> [B,T,HC]
        h_f[:, hr] = res["hf"].T
        c_f[:, hr] = res["cf"]
    return y, h_f, c_f


_CACHE = {}


def kernel(x, W_ih, W_hh, b_ih, b_hh, in_mask, hid_mask, out_mask):
    _patch_libnrt()
    from concourse import bass_utils
    from concourse.bass_interp import get_hw_module

    in_maps = _prep_inputs(
        x, W_ih, W_hh, b_ih, b_hh, in_mask, hid_mask, out_mask
    )

    key = _DT_STR
    if key not in _CACHE:
        nc = build()
        nc.m = get_hw_module(nc.m)
        _CACHE[key] = nc
    nc = _CACHE[key]

    res = bass_utils.run_bass_kernel_spmd(
        nc, in_maps, core_ids=list(range(NC))
    )
    return _assemble(res.results)
